# revision 30
# baseline (speedup 1.0000x reference)
"""HierarchicalAffinityAveraging Trainium2 Bass kernel (v2).

Math (per batch): for lvl in {0,1}: a = softmax(aff[lvl], 9); emb = sum_j
shift_clamped(emb, off_j) * a_j   with off = 3^lvl * {-1,0,1}^2.

Strategy (8 cores = 4 batches x 2 row-halves, full I/O):
  - Device gets RAW (row/col clamp-padded) f16 affinities and bf16 emb;
    all per-offset (row, col) shifts are applied by DMA descriptors and
    matmul column windows -- no host pre-shifting.
  - Per tile: exp(aff_j) on ScalarE; softmax denominator Z via the same
    diagonal-shift matmuls that later do the y-shift sums; r = 1/Z;
    normalized planes a^_j = exp * r_shift (r re-aligned per oy via tiny
    SBUF->SBUF DMA copies into pre-zeroed tiles).  Products W_j = a^_j * e
    on DVE (a few planes on GPSIMD); out = sum_j S_oy^T @ W_j[:, x-window]
    accumulated in PSUM; evacuation is a plain ScalarE copy (already
    normalized).
  - Level-0 output written into level-1's V-tiles (bf16, col-padded);
    level-1 runs the identical pipeline.
  - The short last row-tile (18 / 12 rows) is computed with
    partition-STACKED planes: blocks of (plane j, 20-row slab) stacked to
    ~128 partitions so one matmul sums several planes and the DVE ops use
    full partitions.
"""
import numpy as np
import ml_dtypes

import concourse.bacc as bacc
import concourse.mybir as mybir
import concourse.tile as tile
from concourse.bass_utils import run_bass_kernel_spmd

F32 = mybir.dt.float32
F16 = mybir.dt.float16
BF16 = mybir.dt.bfloat16
AF = mybir.ActivationFunctionType

B, C, H, W = 4, 34, 512, 512
E = 16
NOFF = 9
HALF = H // 2
BASE = [(oy, ox) for oy in (-1, 0, 1) for ox in (-1, 0, 1)]
NCORE = 8
CHG = 4

RA = 268          # aff rows: core rows [-6, 262) clamped
RE = 266          # emb rows: core rows [-4, 262) clamped
WE = 514          # emb cols: [-1, 513) clamped
XV = 518          # v1 col-padded width (halo 3 each side)

# main row tiles (start, P) and the stacked sliver; K = P + 2d
T0 = [(0, 122), (122, 122)]
T1 = [(0, 122), (122, 122)]
SL0 = (244, 18)   # lvl0 sliver: K0 = 20
SL1 = (244, 12)   # lvl1 sliver: K0 = 18
SLIV = {0: [list(range(0, 5)), list(range(5, 9))],     # 5*20=100, 4*20=80
        1: [list(range(0, 7)), list(range(7, 9))]}     # 7*18=126, 2*18=36
V1S = [0, 122, 244]
V1N = [128, 128, 18]
GPJ = (7, 8)      # product planes on gpsimd, level-1 main tiles
GPJ0 = (7, 8)     # product planes on gpsimd, level-0 main tiles
NORM_GP = ()      # oy-groups whose normalize-mul runs on gpsimd
WBUFS = 4         # DVE product-tile ring depth


def _build(reps=1):
    nc = bacc.Bacc("TRN2", target_bir_lowering=False, debug=False, num_devices=NCORE)
    affd = nc.dram_tensor("aff", [2, RA, NOFF, W], F16, kind="ExternalInput")
    e0d = nc.dram_tensor("e0", [RE, E, WE], BF16, kind="ExternalInput")
    smd = nc.dram_tensor("sm", [6, 128, 128], BF16, kind="ExternalInput")
    ssd = nc.dram_tensor("ss", [4, 128, 32], BF16, kind="ExternalInput")
    rfd = nc.dram_tensor("rf", [32, 2, 32], BF16, kind="ExternalInput")
    outd = nc.dram_tensor("out", [HALF, E, W], BF16, kind="ExternalOutput")

    with tile.TileContext(nc) as tc:
        with (
            tc.tile_pool(name="sb", bufs=1) as P1,
            tc.tile_pool(name="db", bufs=2) as P2,
            tc.tile_pool(name="ax", bufs=1) as PX,
            tc.tile_pool(name="af", bufs=2) as PA,
            tc.tile_pool(name="wp", bufs=3) as PW,
            tc.tile_pool(name="se", bufs=1) as PSE,
            tc.tile_pool(name="ps", bufs=2, space="PSUM") as PP,
        ):
            # resident: shift matrices, stacked-sliver matrices, rfix, v1, r-shift
            S = []
            for m in range(6):
                st = P1.tile([128, 128], BF16, tag=f"S{m}")
                nc.scalar.dma_start(st[:], smd[m])
                S.append(st)
            SS = []
            for k in range(4):
                st = P1.tile([128, 32], BF16, tag=f"SS{k}")
                nc.scalar.dma_start(st[:], ssd[k])
                SS.append(st)
            rft = P1.tile([32, 2, 32], BF16, tag="rf")
            nc.scalar.dma_start(rft[:], rfd[:])
            v1 = []
            for t in range(3):
                vt = P1.tile([V1N[t], E, XV], BF16, tag=f"v1_{t}")
                v1.append(vt)
            rm = []
            for t in range(3):
                rt_ = P1.tile([128, 512], BF16, tag=f"rm{t}")
                nc.vector.memset(rt_[:], 0.0)
                rm.append(rt_)
            rst = []
            for s in range(2):
                rt_ = P1.tile([128, 512], BF16, tag=f"rst{s}")
                nc.vector.memset(rt_[:], 0.0)
                rst.append(rt_)

            # e-tile column slice start for offset ox:
            #   lvl0 (e_pad, col c == x-1):  x+ox -> c = ox+1, ox in {-1,0,1} -> {0,1,2}
            #   lvl1 (v1, col c == x-3):     x+3ox -> c = 3ox+3 -> {0,3,6}
            def ecol0(lvl, ox):
                d = 3 ** lvl
                return ox * d + (1 if lvl == 0 else 3)

            def evac0(a, P, g, ops, dve=False):
                # lvl0: write psum rows [a, a+P) (O0-local) -> v1 segments,
                # all CHG channels of group g in one op per segment
                ch0 = g * CHG
                for t in range(3):
                    s0, n0 = V1S[t], V1N[t]
                    lo, hi = max(a, s0), min(a + P, s0 + n0)
                    if lo >= hi:
                        continue
                    dst = v1[t][lo - s0: hi - s0, ch0: ch0 + CHG, 3: 3 + 512]
                    src = ops[lo - a: hi - a, :, :]
                    if lo - s0 == lo - a:
                        if dve:
                            nc.vector.tensor_copy(dst, src)
                        else:
                            nc.scalar.activation(dst, src, AF.Copy)
                    else:
                        tmp = P2.tile([hi - lo, CHG, 512], BF16, tag="seg")
                        if dve:
                            nc.vector.tensor_copy(tmp[:], src)
                        else:
                            nc.scalar.activation(tmp[:], src, AF.Copy)
                        nc.scalar.dma_start(dst, tmp[:])

            def evac1(a, P, g, ops, dve=False):
                ob = P2.tile([P, CHG, 512], BF16, tag="ob")
                if dve:
                    nc.vector.tensor_copy(ob[:], ops[:])
                else:
                    nc.scalar.activation(ob[:], ops[:], AF.Copy)
                # one DMA: contiguous (row, ch, x) layout
                dst = outd[a: a + P, g * CHG:(g + 1) * CHG, :]
                nc.scalar.dma_start(dst, ob[:])
                return

            def zA_main(lvl, a, P):
                # aff DMA + exp only (issued 2 tiles ahead)
                d = 3 ** lvl
                K = P + 2 * d
                rowbase = a + (3 if lvl == 0 else 6)
                af = PA.tile([K, NOFF, 512], F16, tag="af")
                nc.sync.dma_start(
                    af[:], affd[lvl, rowbase: rowbase + K, :, :],
                )
                ax = PX.tile([K, NOFF, 512], BF16, tag="ax", bufs=2)
                for t in range(3):
                    nc.scalar.activation(
                        ax[:, 3 * t: 3 * t + 3, :], af[:, 3 * t: 3 * t + 3, :],
                        AF.Exp)
                return ax

            def zB_main(lvl, a, P, ax):
                # Z matmuls, r = 1/Z, aligned r copies, normalized planes
                d = 3 ** lvl
                K = P + 2 * d
                zb = PP.tile([P, CHG, 512], F32, tag="ops")
                for j, (oy, ox) in enumerate(BASE):
                    t = oy + 1
                    nc.tensor.matmul(
                        zb[:, 0, :], S[lvl * 3 + t][0:K, 0:P], ax[:, j, :],
                        start=(j == 0), stop=(j == NOFF - 1),
                    )
                r32 = P2.tile([P, 512], F32, tag="r32", bufs=1)
                nc.vector.reciprocal_approx_fast(r32[:], zb[:, 0, :])
                rb = P2.tile([P, 512], BF16, tag="rb")
                nc.vector.tensor_copy(rb[:], r32[:])
                for t in range(3):
                    nc.scalar.dma_start(rm[t][t * d: t * d + P, :], rb[:])
                axn = PX.tile([K, NOFF, 512], BF16, tag="axn", bufs=2)
                for t in range(3):
                    src_r = rm[t][0:K].unsqueeze(1).broadcast_to([K, 3, 512])
                    eng = nc.gpsimd if t in NORM_GP else nc.vector
                    eng.tensor_mul(
                        axn[:, 3 * t: 3 * t + 3, :], ax[:, 3 * t: 3 * t + 3, :],
                        src_r,
                    )
                return axn

            def grp_main(lvl, a, P, axn, injectB=None, injectA=None,
                         injectE=None, postg=None):
                d = 3 ** lvl
                K = P + 2 * d
                # --- products + shift-sum (e loaded per channel group) ---
                for g in range(E // CHG):
                    if g == 1 and injectB is not None:
                        injectB()
                    if g == 2 and injectA is not None:
                        injectA()
                    if g == 3 and injectE is not None:
                        injectE()
                    if lvl == 0:
                        et = P2.tile([K, CHG, WE], BF16, tag="et", bufs=2)
                        esrc = e0d[a: a + K, g * CHG:(g + 1) * CHG, :]
                        nc.sync.dma_start(et[:], esrc)
                        ecs = 0
                    else:
                        t_ = V1S.index(a)
                        et = v1[t_][0:K]
                        ecs = g * CHG
                    ops = PP.tile([P, CHG, 512], F32, tag="ops")
                    for j, (oy, ox) in enumerate(BASE):
                        t = oy + 1
                        c0 = ecol0(lvl, ox)
                        gp = j in (GPJ0 if lvl == 0 else GPJ)
                        wt = PW.tile([K, CHG, 512], BF16, tag="WP" if gp else "W", bufs=2 if gp else WBUFS)
                        src_a = axn[:, j, :].unsqueeze(1).broadcast_to([K, CHG, 512])
                        eng = nc.gpsimd if gp else nc.vector
                        eng.tensor_mul(
                            wt[:], src_a,
                            et[:, ecs: ecs + CHG, c0: c0 + 512],
                        )
                        for i in range(CHG):
                            nc.tensor.matmul(
                                ops[:, i, :], S[lvl * 3 + t][0:K, 0:P],
                                wt[:, i, :],
                                start=(j == 0), stop=(j == NOFF - 1),
                            )
                    if lvl == 0:
                        evac0(a, P, g, ops)
                    else:
                        evac1(a, P, g, ops)
                    if postg is not None:
                        postg(g)

            def zA_sliver(lvl, a, P):
                d = 3 ** lvl
                K0 = P + 2 * d
                rowbase = a + (3 if lvl == 0 else 6)
                stacks = SLIV[lvl]
                sax = []
                for s, js in enumerate(stacks):
                    n = len(js) * K0
                    saf = PA.tile([n, 512], F16, tag=f"saf{s}")
                    nc.sync.dma_start(
                        saf[:],
                        affd[lvl, rowbase: rowbase + K0,
                             js[0]: js[-1] + 1, :].transpose([1, 0, 2]),
                    )
                    x = PX.tile([n, 512], BF16, tag=f"sax{s}")
                    nc.scalar.activation(x[:], saf[:], AF.Exp)
                    sax.append(x)
                return sax

            def zB_sliver(lvl, a, P, sax):
                d = 3 ** lvl
                K0 = P + 2 * d
                stacks = SLIV[lvl]
                zb = PP.tile([P, CHG, 512], F32, tag="ops")
                for s, js in enumerate(stacks):
                    n = len(js) * K0
                    nc.tensor.matmul(
                        zb[:, 0, :], SS[lvl * 2 + s][0:n, 0:P], sax[s][:],
                        start=(s == 0), stop=(s == len(stacks) - 1),
                    )
                r32 = P2.tile([P, 512], F32, tag="r32", bufs=1)
                nc.vector.reciprocal_approx_fast(r32[:], zb[:, 0, :])
                rb = P2.tile([P, 512], BF16, tag="rb")
                nc.vector.tensor_copy(rb[:], r32[:])
                saxn = []
                for s, js in enumerate(stacks):
                    n = len(js) * K0
                    for b, j in enumerate(js):
                        t = BASE[j][0] + 1
                        nc.scalar.dma_start(
                            rst[s][b * K0 + t * d: b * K0 + t * d + P, :],
                            rb[:])
                    xn = PX.tile([n, 512], BF16, tag=f"saxn{s}")
                    nc.vector.tensor_mul(xn[:], sax[s][:], rst[s][0:n, :])
                    saxn.append(xn)
                return saxn

            EH = E // 2

            def load_se(lvl, a, s, half):
                # stacked e for sliver tiles (half the channels at a time)
                d = 3 ** lvl
                K0 = (SL0 if lvl == 0 else SL1)[1] + 2 * d
                js = SLIV[lvl][s]
                n = len(js) * K0
                st_ = PSE.tile([n, EH, 512], BF16, tag=f"se{s}")
                ch0 = half * EH
                for b, j in enumerate(js):
                    ox = BASE[j][1]
                    c0 = ecol0(lvl, ox)
                    if lvl == 0:
                        esrc = e0d[a: a + K0, ch0: ch0 + EH, c0: c0 + 512]
                        nc.sync.dma_start(
                            st_[b * K0:(b + 1) * K0], esrc)
                    else:
                        t_ = V1S.index(a)
                        nc.sync.dma_start(
                            st_[b * K0:(b + 1) * K0],
                            v1[t_][0:K0, ch0: ch0 + EH, c0: c0 + 512],
                        )
                return st_

            def grp_sliver(lvl, a, P, saxn, injectB=None, injectA=None,
                           postg=None, se_pre=None):
                d = 3 ** lvl
                K0 = P + 2 * d
                stacks = SLIV[lvl]

                def src_a_s(s, n):
                    return saxn[s][:].unsqueeze(1).broadcast_to([n, CHG, 512])

                # --- products + stacked shift-sum ---
                se = se_pre
                for g in range(E // CHG):
                    if g == 1 and injectB is not None:
                        injectB()
                    if g == 2 and injectA is not None:
                        injectA()
                    if g == 2 or (g == 0 and se is None):
                        se = [load_se(lvl, a, 0, g // 2),
                              load_se(lvl, a, 1, g // 2)]
                    ops = PP.tile([P, CHG, 512], F32, tag="ops")
                    for s, js in enumerate(stacks):
                        n = len(js) * K0
                        wt = PW.tile([n, CHG, 512], BF16,
                                     tag="SWP" if s == 1 else "SW", bufs=2)
                        seng = nc.vector
                        gc = (g % 2) * CHG
                        seng.tensor_mul(
                            wt[:], src_a_s(s, n),
                            se[s][:, gc: gc + CHG, :])
                        for i in range(CHG):
                            nc.tensor.matmul(
                                ops[:, i, :], SS[lvl * 2 + s][0:n, 0:P],
                                wt[:, i, :],
                                start=(s == 0), stop=(s == len(stacks) - 1),
                            )
                    if lvl == 0:
                        evac0(a, P, g, ops)
                    else:
                        evac1(a, P, g, ops)
                    if postg is not None:
                        postg(g)

            def rfix_g(vt, kk, rr, g):
                # edge-row fix: v1[0][0:3] / v1[2][15:18] replicate-or-identity
                n = V1N[vt]
                lo = 0 if rr == 0 else n - 3
                fps = PP.tile([3, CHG, 512], F32, tag="ops")
                tmpf = P2.tile([3, CHG, 512], BF16, tag="fxs", bufs=1)
                for i in range(CHG):
                    nc.tensor.matmul(
                        fps[:, i, :], rft[0:kk, rr, 0:3],
                        v1[vt][0:kk, g * CHG + i, 3: 3 + 512],
                        start=True, stop=True,
                    )
                nc.scalar.activation(tmpf[:], fps[:], AF.Copy)
                nc.scalar.dma_start(
                    v1[vt][lo: lo + 3, g * CHG:(g + 1) * CHG, 3: 3 + 512],
                    tmpf[:])

            def halo_g(t, g):
                # x-halo pad of v1 (replicate col 3 -> 0..2, col 514 -> 515..517)
                n = V1N[t]
                c0 = g * CHG
                src = v1[t][:, c0: c0 + CHG, 3:4].broadcast_to([n, CHG, 3])
                nc.vector.tensor_copy(v1[t][:, c0: c0 + CHG, 0:3], src)
                src2 = v1[t][:, c0: c0 + CHG, 514:515].broadcast_to([n, CHG, 3])
                nc.vector.tensor_copy(v1[t][:, c0: c0 + CHG, 515:518], src2)

            # software-pipelined tile sequence: zA (aff DMA + exp) issued two
            # tiles ahead, zB (Z matmuls, 1/Z, normalized planes) one tile
            # ahead.  lvl1-tile0 is interleaved BEFORE the lvl0 sliver (its
            # v1[0] input is complete after lvl0-tile1 + per-group
            # rfix/halo), hiding the level transition; the pipeline also
            # runs across reps.
            tiles = [(0, "m", *T0[0]), (0, "m", *T0[1]), (1, "m", *T1[0]),
                     (0, "s", *SL0), (1, "m", *T1[1]), (1, "s", *SL1)]

            def postg_for(i):
                im = i % ntile
                if im == 0:
                    def f(g):
                        rfix_g(0, 4, 0, g)
                    return f
                if im == 1:
                    def f(g):
                        halo_g(0, g)
                    return f
                if im == 3:
                    def f(g):
                        halo_g(1, g)
                        rfix_g(2, 18, 1, g)
                        halo_g(2, g)
                    return f
                return None

            ntile = len(tiles)
            all_tiles = [tiles[i % ntile] for i in range(ntile * reps)]

            def do_zA(i):
                lvl, kind, a, P = all_tiles[i]
                return (zA_main if kind == "m" else zA_sliver)(lvl, a, P)

            def do_zB(i):
                lvl, kind, a, P = all_tiles[i]
                return (zB_main if kind == "m" else zB_sliver)(
                    lvl, a, P, ctxA.pop(i))

            ctxA = {0: do_zA(0), 1: do_zA(1)}
            ctx = {0: do_zB(0)}

            def mk_injB(nxt):
                def f():
                    ctx[nxt] = do_zB(nxt)
                return f

            def mk_injA(nxt):
                def f():
                    ctxA[nxt] = do_zA(nxt)
                return f

            se_pre = {}

            def mk_injE(nxt):
                lvl_n, _, a_n, _ = all_tiles[nxt]

                def f():
                    se_pre[nxt] = [load_se(lvl_n, a_n, 0, 0),
                                   load_se(lvl_n, a_n, 1, 0)]
                return f

            for i, (lvl, kind, a, P) in enumerate(all_tiles):
                injB = mk_injB(i + 1) if i + 1 < len(all_tiles) else None
                injA = mk_injA(i + 2) if i + 2 < len(all_tiles) else None
                injE = None
                if i + 1 < len(all_tiles) and all_tiles[i + 1][1] == "s":
                    injE = mk_injE(i + 1)
                pg = postg_for(i)
                if kind == "m":
                    grp_main(lvl, a, P, ctx[i], injectB=injB, injectA=injA,
                             injectE=injE, postg=pg)
                else:
                    grp_sliver(lvl, a, P, ctx[i], injectB=injB, injectA=injA,
                               postg=pg, se_pre=se_pre.pop(i, None))

    nc.compile()
    return nc


_NC_CACHE = None


def _get_nc():
    global _NC_CACHE
    if _NC_CACHE is None:
        _NC_CACHE = _build()
    return _NC_CACHE


def _host_prep(inp):
    """Build per-core input dicts from the full (B,C,H,W) f32 array."""
    aff = inp[:, : 2 * NOFF]
    emb = inp[:, 2 * NOFF:]

    # shift matrices: S[lvl*3+t][p + t*d, p] = 1
    sm = np.zeros((6, 128, 128), dtype=np.float32)
    for lvl in range(2):
        d = 3 ** lvl
        for t in range(3):
            for p in range(128):
                if p + t * d < 128:
                    sm[lvl * 3 + t, p + t * d, p] = 1.0
    sm = sm.astype(ml_dtypes.bfloat16)

    # stacked sliver matrices: SS[lvl*2+s][b*K0 + p + t_b*d, p] = 1
    ss = np.zeros((4, 128, 32), dtype=np.float32)
    for lvl, (a_, P_) in ((0, SL0), (1, SL1)):
        d = 3 ** lvl
        K0 = P_ + 2 * d
        for s, js in enumerate(SLIV[lvl]):
            for b, j in enumerate(js):
                t = BASE[j][0] + 1
                for p in range(P_):
                    ss[lvl * 2 + s, b * K0 + p + t * d, p] = 1.0
    ss = ss.astype(ml_dtypes.bfloat16)

    cols_e = np.clip(np.arange(-1, 513), 0, W - 1)

    maps = []
    for b in range(B):
        affb16 = aff[b].astype(np.float16)
        embb16 = emb[b].astype(ml_dtypes.bfloat16)
        for h in range(2):
            base = h * HALF
            # affp[lvl, j, r] = aff ch 9*lvl+j at core row (r - 6 - t_j * d)
            affp = np.empty((2, NOFF, RA, W), np.float16)
            for lvl in range(2):
                d = 3 ** lvl
                for t in range(3):
                    rt = np.clip(base + np.arange(-6, 262) - t * d, 0, H - 1)
                    affp[lvl, 3 * t: 3 * t + 3] = affb16[
                        NOFF * lvl + 3 * t: NOFF * lvl + 3 * t + 3][:, rt, :]
            affp = np.ascontiguousarray(affp.transpose(0, 2, 1, 3))
            re = np.clip(base + np.arange(-4, 262), 0, H - 1)
            ep = np.ascontiguousarray(
                embb16[:, re][:, :, cols_e].transpose(1, 0, 2))
            rf = np.zeros((32, 2, 32), np.float32)
            for p in range(3):
                rf[3 if h == 0 else p, 0, p] = 1.0
                rf[14 if h == 1 else 15 + p, 1, p] = 1.0
            maps.append({"aff": affp, "e0": ep, "sm": sm, "ss": ss,
                         "rf": rf.astype(ml_dtypes.bfloat16)})
    return maps


def kernel(input):
    inp = np.asarray(input, dtype=np.float32)
    nc = _get_nc()
    maps = _host_prep(inp)
    res = run_bass_kernel_spmd(nc, maps, core_ids=list(range(NCORE)))
    full = np.empty((B, E, H, W), dtype=np.float32)
    k = 0
    for b in range(B):
        for h in range(2):
            out = np.asarray(res.results[k]["out"])  # [HALF, E, W]
            full[b, :, h * HALF: (h + 1) * HALF, :] = out.transpose(1, 0, 2)
            k += 1
    return full



# revision 33
# speedup vs baseline: 1.0996x; 1.0996x over previous
"""HierarchicalAffinityAveraging Trainium2 Bass kernel (v2).

Math (per batch): for lvl in {0,1}: a = softmax(aff[lvl], 9); emb = sum_j
shift_clamped(emb, off_j) * a_j   with off = 3^lvl * {-1,0,1}^2.

Strategy (8 cores = 4 batches x 2 row-halves, full I/O):
  - Device gets RAW (row/col clamp-padded) f16 affinities and bf16 emb;
    all per-offset (row, col) shifts are applied by DMA descriptors and
    matmul column windows -- no host pre-shifting.
  - Per tile: exp(aff_j) on ScalarE; softmax denominator Z via the same
    diagonal-shift matmuls that later do the y-shift sums; r = 1/Z;
    normalized planes a^_j = exp * r_shift (r re-aligned per oy via tiny
    SBUF->SBUF DMA copies into pre-zeroed tiles).  Products W_j = a^_j * e
    on DVE (a few planes on GPSIMD); out = sum_j S_oy^T @ W_j[:, x-window]
    accumulated in PSUM; evacuation is a plain ScalarE copy (already
    normalized).
  - Level-0 output written into level-1's V-tiles (bf16, col-padded);
    level-1 runs the identical pipeline.
  - The short last row-tile (18 / 12 rows) is computed with
    partition-STACKED planes: blocks of (plane j, 20-row slab) stacked to
    ~128 partitions so one matmul sums several planes and the DVE ops use
    full partitions.
"""
import numpy as np
import ml_dtypes

import concourse.bacc as bacc
import concourse.mybir as mybir
import concourse.tile as tile
from concourse.bass_utils import run_bass_kernel_spmd

F32 = mybir.dt.float32
F16 = mybir.dt.float16
BF16 = mybir.dt.bfloat16
AF = mybir.ActivationFunctionType

B, C, H, W = 4, 34, 512, 512
E = 16
NOFF = 9
HALF = H // 2
BASE = [(oy, ox) for oy in (-1, 0, 1) for ox in (-1, 0, 1)]
NCORE = 8
CHG = 4

RA = 268          # aff rows: core rows [-6, 262) clamped
RE = 266          # emb rows: core rows [-4, 262) clamped
WE = 514          # emb cols: [-1, 513) clamped
XV = 518          # v1 col-padded width (halo 3 each side)

# main row tiles (start, P) and the stacked sliver; K = P + 2d
T0 = [(0, 122), (122, 122)]
T1 = [(0, 122), (122, 122)]
SL0 = (244, 18)   # lvl0 sliver: K0 = 20
SL1 = (244, 12)   # lvl1 sliver: K0 = 18
SLIV = {0: [list(range(0, 5)), list(range(5, 9))],     # 5*20=100, 4*20=80
        1: [list(range(0, 7)), list(range(7, 9))]}     # 7*18=126, 2*18=36
V1S = [0, 122, 244]
V1N = [128, 128, 18]
GPJ = (7, 8)      # product planes on gpsimd, level-1 main tiles
GPJ0 = (7, 8)     # product planes on gpsimd, level-0 main tiles
NORM_GP = ()      # oy-groups whose normalize-mul runs on gpsimd
WBUFS = 5         # DVE product-tile ring depth
WPBUFS = 4        # gpsimd product-tile ring depth


def _build(reps=1):
    nc = bacc.Bacc("TRN2", target_bir_lowering=False, debug=False, num_devices=NCORE)
    affd = nc.dram_tensor("aff", [2, RA, NOFF, W], F16, kind="ExternalInput")
    e0d = nc.dram_tensor("e0", [RE, E, WE], BF16, kind="ExternalInput")
    smd = nc.dram_tensor("sm", [6, 128, 128], BF16, kind="ExternalInput")
    ssd = nc.dram_tensor("ss", [4, 128, 32], BF16, kind="ExternalInput")
    rfd = nc.dram_tensor("rf", [32, 2, 32], BF16, kind="ExternalInput")
    outd = nc.dram_tensor("out", [HALF, E, W], BF16, kind="ExternalOutput")

    with tile.TileContext(nc) as tc:
        with (
            tc.tile_pool(name="sb", bufs=1) as P1,
            tc.tile_pool(name="db", bufs=2) as P2,
            tc.tile_pool(name="ax", bufs=1) as PX,
            tc.tile_pool(name="af", bufs=2) as PA,
            tc.tile_pool(name="wp", bufs=3) as PW,
            tc.tile_pool(name="se", bufs=1) as PSE,
            tc.tile_pool(name="ps", bufs=2, space="PSUM") as PP,
        ):
            # resident: shift matrices, stacked-sliver matrices, rfix, v1, r-shift
            S = []
            for m in range(6):
                st = P1.tile([128, 128], BF16, tag=f"S{m}")
                nc.scalar.dma_start(st[:], smd[m])
                S.append(st)
            SS = []
            for k in range(4):
                st = P1.tile([128, 32], BF16, tag=f"SS{k}")
                nc.scalar.dma_start(st[:], ssd[k])
                SS.append(st)
            rft = P1.tile([32, 2, 32], BF16, tag="rf")
            nc.scalar.dma_start(rft[:], rfd[:])
            v1 = []
            for t in range(3):
                vt = P1.tile([V1N[t], E, XV], BF16, tag=f"v1_{t}")
                v1.append(vt)
            rm = []
            for t in range(3):
                rt_ = P1.tile([128, 512], BF16, tag=f"rm{t}")
                nc.vector.memset(rt_[:], 0.0)
                rm.append(rt_)
            rst = []
            for s in range(2):
                rt_ = P1.tile([128, 512], BF16, tag=f"rst{s}")
                nc.vector.memset(rt_[:], 0.0)
                rst.append(rt_)

            # e-tile column slice start for offset ox:
            #   lvl0 (e_pad, col c == x-1):  x+ox -> c = ox+1, ox in {-1,0,1} -> {0,1,2}
            #   lvl1 (v1, col c == x-3):     x+3ox -> c = 3ox+3 -> {0,3,6}
            def ecol0(lvl, ox):
                d = 3 ** lvl
                return ox * d + (1 if lvl == 0 else 3)

            def evac0(a, P, g, ops, dve=False):
                # lvl0: write psum rows [a, a+P) (O0-local) -> v1 segments,
                # all CHG channels of group g in one op per segment
                ch0 = g * CHG
                for t in range(3):
                    s0, n0 = V1S[t], V1N[t]
                    lo, hi = max(a, s0), min(a + P, s0 + n0)
                    if lo >= hi:
                        continue
                    dst = v1[t][lo - s0: hi - s0, ch0: ch0 + CHG, 3: 3 + 512]
                    src = ops[lo - a: hi - a, :, :]
                    if lo - s0 == lo - a:
                        if dve:
                            nc.vector.tensor_copy(dst, src)
                        else:
                            nc.scalar.activation(dst, src, AF.Copy)
                    else:
                        tmp = P2.tile([hi - lo, CHG, 512], BF16, tag="seg")
                        if dve:
                            nc.vector.tensor_copy(tmp[:], src)
                        else:
                            nc.scalar.activation(tmp[:], src, AF.Copy)
                        nc.scalar.dma_start(dst, tmp[:])

            def evac1(a, P, g, ops, dve=False):
                ob = P2.tile([P, CHG, 512], BF16, tag="ob")
                if dve:
                    nc.vector.tensor_copy(ob[:], ops[:])
                else:
                    nc.scalar.activation(ob[:], ops[:], AF.Copy)
                # one DMA: contiguous (row, ch, x) layout
                dst = outd[a: a + P, g * CHG:(g + 1) * CHG, :]
                nc.scalar.dma_start(dst, ob[:])
                return

            def zA_main(lvl, a, P):
                # aff DMA + exp only (issued 2 tiles ahead)
                d = 3 ** lvl
                K = P + 2 * d
                rowbase = a + (3 if lvl == 0 else 6)
                af = PA.tile([K, NOFF, 512], F16, tag="af")
                nc.sync.dma_start(
                    af[:], affd[lvl, rowbase: rowbase + K, :, :],
                )
                ax = PX.tile([K, NOFF, 512], BF16, tag="ax", bufs=1)
                for t in range(3):
                    nc.scalar.activation(
                        ax[:, 3 * t: 3 * t + 3, :], af[:, 3 * t: 3 * t + 3, :],
                        AF.Exp)
                return ax

            def zB_main(lvl, a, P, ax):
                # Z matmuls, r = 1/Z, aligned r copies, normalized planes
                d = 3 ** lvl
                K = P + 2 * d
                zb = PP.tile([P, CHG, 512], F32, tag="ops")
                for j, (oy, ox) in enumerate(BASE):
                    t = oy + 1
                    nc.tensor.matmul(
                        zb[:, 0, :], S[lvl * 3 + t][0:K, 0:P], ax[:, j, :],
                        start=(j == 0), stop=(j == NOFF - 1),
                    )
                r32 = P2.tile([P, 512], F32, tag="r32", bufs=1)
                nc.vector.reciprocal_approx_fast(r32[:], zb[:, 0, :])
                rb = P2.tile([P, 512], BF16, tag="rb", bufs=1)
                nc.vector.tensor_copy(rb[:], r32[:])
                for t in range(3):
                    nc.scalar.dma_start(rm[t][t * d: t * d + P, :], rb[:])
                axn = PX.tile([K, NOFF, 512], BF16, tag="axn", bufs=2)
                for t in range(3):
                    src_r = rm[t][0:K].unsqueeze(1).broadcast_to([K, 3, 512])
                    eng = nc.gpsimd if t in NORM_GP else nc.vector
                    eng.tensor_mul(
                        axn[:, 3 * t: 3 * t + 3, :], ax[:, 3 * t: 3 * t + 3, :],
                        src_r,
                    )
                return axn

            def grp_main(lvl, a, P, axn, injectB=None, injectA=None,
                         injectE=None, postg=None):
                d = 3 ** lvl
                K = P + 2 * d
                # --- products + shift-sum (e loaded per channel group) ---
                for g in range(E // CHG):
                    if g == 1 and injectA is not None:
                        injectA()
                    if g == 2 and injectB is not None:
                        injectB()
                    if g == 3 and injectE is not None:
                        injectE()
                    if lvl == 0:
                        et = P2.tile([K, CHG, WE], BF16, tag="et", bufs=2)
                        esrc = e0d[a: a + K, g * CHG:(g + 1) * CHG, :]
                        nc.sync.dma_start(et[:], esrc)
                        ecs = 0
                    else:
                        t_ = V1S.index(a)
                        et = v1[t_][0:K]
                        ecs = g * CHG
                    ops = PP.tile([P, CHG, 512], F32, tag="ops")
                    for j, (oy, ox) in enumerate(BASE):
                        t = oy + 1
                        c0 = ecol0(lvl, ox)
                        gp = j in (GPJ0 if lvl == 0 else GPJ)
                        wt = PW.tile([K, CHG, 512], BF16, tag="WP" if gp else "W", bufs=WPBUFS if gp else WBUFS)
                        src_a = axn[:, j, :].unsqueeze(1).broadcast_to([K, CHG, 512])
                        eng = nc.gpsimd if gp else nc.vector
                        eng.tensor_mul(
                            wt[:], src_a,
                            et[:, ecs: ecs + CHG, c0: c0 + 512],
                        )
                        for i in range(CHG):
                            nc.tensor.matmul(
                                ops[:, i, :], S[lvl * 3 + t][0:K, 0:P],
                                wt[:, i, :],
                                start=(j == 0), stop=(j == NOFF - 1),
                            )
                    if lvl == 0:
                        evac0(a, P, g, ops)
                    else:
                        evac1(a, P, g, ops)
                    if postg is not None:
                        postg(g)

            def zA_sliver(lvl, a, P):
                d = 3 ** lvl
                K0 = P + 2 * d
                rowbase = a + (3 if lvl == 0 else 6)
                stacks = SLIV[lvl]
                sax = []
                for s, js in enumerate(stacks):
                    n = len(js) * K0
                    saf = PA.tile([n, 512], F16, tag=f"saf{s}")
                    nc.sync.dma_start(
                        saf[:],
                        affd[lvl, rowbase: rowbase + K0,
                             js[0]: js[-1] + 1, :].transpose([1, 0, 2]),
                    )
                    x = PX.tile([n, 512], BF16, tag=f"sax{s}")
                    nc.scalar.activation(x[:], saf[:], AF.Exp)
                    sax.append(x)
                return sax

            def zB_sliver(lvl, a, P, sax):
                d = 3 ** lvl
                K0 = P + 2 * d
                stacks = SLIV[lvl]
                zb = PP.tile([P, CHG, 512], F32, tag="ops")
                for s, js in enumerate(stacks):
                    n = len(js) * K0
                    nc.tensor.matmul(
                        zb[:, 0, :], SS[lvl * 2 + s][0:n, 0:P], sax[s][:],
                        start=(s == 0), stop=(s == len(stacks) - 1),
                    )
                r32 = P2.tile([P, 512], F32, tag="r32", bufs=1)
                nc.vector.reciprocal_approx_fast(r32[:], zb[:, 0, :])
                rb = P2.tile([P, 512], BF16, tag="rb", bufs=1)
                nc.vector.tensor_copy(rb[:], r32[:])
                saxn = []
                for s, js in enumerate(stacks):
                    n = len(js) * K0
                    for b, j in enumerate(js):
                        t = BASE[j][0] + 1
                        nc.scalar.dma_start(
                            rst[s][b * K0 + t * d: b * K0 + t * d + P, :],
                            rb[:])
                    xn = PX.tile([n, 512], BF16, tag=f"saxn{s}")
                    nc.vector.tensor_mul(xn[:], sax[s][:], rst[s][0:n, :])
                    saxn.append(xn)
                return saxn

            EH = E // 2

            def load_se(lvl, a, s, half):
                # stacked e for sliver tiles (half the channels at a time)
                d = 3 ** lvl
                K0 = (SL0 if lvl == 0 else SL1)[1] + 2 * d
                js = SLIV[lvl][s]
                n = len(js) * K0
                st_ = PSE.tile([n, EH, 512], BF16, tag=f"se{s}")
                ch0 = half * EH
                for b, j in enumerate(js):
                    ox = BASE[j][1]
                    c0 = ecol0(lvl, ox)
                    if lvl == 0:
                        esrc = e0d[a: a + K0, ch0: ch0 + EH, c0: c0 + 512]
                        nc.sync.dma_start(
                            st_[b * K0:(b + 1) * K0], esrc)
                    else:
                        t_ = V1S.index(a)
                        nc.sync.dma_start(
                            st_[b * K0:(b + 1) * K0],
                            v1[t_][0:K0, ch0: ch0 + EH, c0: c0 + 512],
                        )
                return st_

            def grp_sliver(lvl, a, P, saxn, injectB=None, injectA=None,
                           postg=None, se_pre=None):
                d = 3 ** lvl
                K0 = P + 2 * d
                stacks = SLIV[lvl]

                def src_a_s(s, n):
                    return saxn[s][:].unsqueeze(1).broadcast_to([n, CHG, 512])

                # --- products + stacked shift-sum ---
                se = se_pre
                for g in range(E // CHG):
                    if g == 1 and injectA is not None:
                        injectA()
                    if g == 2 and injectB is not None:
                        injectB()
                    if g == 2 or (g == 0 and se is None):
                        se = [load_se(lvl, a, 0, g // 2),
                              load_se(lvl, a, 1, g // 2)]
                    ops = PP.tile([P, CHG, 512], F32, tag="ops")
                    for s, js in enumerate(stacks):
                        n = len(js) * K0
                        wt = PW.tile([n, CHG, 512], BF16,
                                     tag="SWP" if s == 1 else "SW", bufs=2)
                        seng = nc.vector
                        gc = (g % 2) * CHG
                        seng.tensor_mul(
                            wt[:], src_a_s(s, n),
                            se[s][:, gc: gc + CHG, :])
                        for i in range(CHG):
                            nc.tensor.matmul(
                                ops[:, i, :], SS[lvl * 2 + s][0:n, 0:P],
                                wt[:, i, :],
                                start=(s == 0), stop=(s == len(stacks) - 1),
                            )
                    if lvl == 0:
                        evac0(a, P, g, ops)
                    else:
                        evac1(a, P, g, ops)
                    if postg is not None:
                        postg(g)

            def rfix_g(vt, kk, rr, g):
                # edge-row fix: v1[0][0:3] / v1[2][15:18] replicate-or-identity
                n = V1N[vt]
                lo = 0 if rr == 0 else n - 3
                fps = PP.tile([3, CHG, 512], F32, tag="ops")
                tmpf = P2.tile([3, CHG, 512], BF16, tag="fxs", bufs=1)
                for i in range(CHG):
                    nc.tensor.matmul(
                        fps[:, i, :], rft[0:kk, rr, 0:3],
                        v1[vt][0:kk, g * CHG + i, 3: 3 + 512],
                        start=True, stop=True,
                    )
                nc.scalar.activation(tmpf[:], fps[:], AF.Copy)
                nc.scalar.dma_start(
                    v1[vt][lo: lo + 3, g * CHG:(g + 1) * CHG, 3: 3 + 512],
                    tmpf[:])

            def halo_g(t, g):
                # x-halo pad of v1 (replicate col 3 -> 0..2, col 514 -> 515..517)
                n = V1N[t]
                c0 = g * CHG
                src = v1[t][:, c0: c0 + CHG, 3:4].broadcast_to([n, CHG, 3])
                nc.vector.tensor_copy(v1[t][:, c0: c0 + CHG, 0:3], src)
                src2 = v1[t][:, c0: c0 + CHG, 514:515].broadcast_to([n, CHG, 3])
                nc.vector.tensor_copy(v1[t][:, c0: c0 + CHG, 515:518], src2)

            # software-pipelined tile sequence: zA (aff DMA + exp) issued two
            # tiles ahead, zB (Z matmuls, 1/Z, normalized planes) one tile
            # ahead.  lvl1-tile0 is interleaved BEFORE the lvl0 sliver (its
            # v1[0] input is complete after lvl0-tile1 + per-group
            # rfix/halo), hiding the level transition; the pipeline also
            # runs across reps.
            tiles = [(0, "m", *T0[0]), (0, "m", *T0[1]), (1, "m", *T1[0]),
                     (0, "s", *SL0), (1, "m", *T1[1]), (1, "s", *SL1)]

            def postg_for(i):
                im = i % ntile
                if im == 0:
                    def f(g):
                        rfix_g(0, 4, 0, g)
                    return f
                if im == 1:
                    def f(g):
                        halo_g(0, g)
                    return f
                if im == 3:
                    def f(g):
                        halo_g(1, g)
                        rfix_g(2, 18, 1, g)
                        halo_g(2, g)
                    return f
                return None

            ntile = len(tiles)
            all_tiles = [tiles[i % ntile] for i in range(ntile * reps)]

            def do_zA(i):
                lvl, kind, a, P = all_tiles[i]
                return (zA_main if kind == "m" else zA_sliver)(lvl, a, P)

            def do_zB(i):
                lvl, kind, a, P = all_tiles[i]
                return (zB_main if kind == "m" else zB_sliver)(
                    lvl, a, P, ctxA.pop(i))

            ctxA = {0: do_zA(0)}
            ctx = {0: do_zB(0)}

            def mk_injB(nxt):
                def f():
                    ctx[nxt] = do_zB(nxt)
                return f

            def mk_injA(nxt):
                def f():
                    ctxA[nxt] = do_zA(nxt)
                return f

            se_pre = {}

            def mk_injE(nxt):
                lvl_n, _, a_n, _ = all_tiles[nxt]

                def f():
                    se_pre[nxt] = [load_se(lvl_n, a_n, 0, 0),
                                   load_se(lvl_n, a_n, 1, 0)]
                return f

            for i, (lvl, kind, a, P) in enumerate(all_tiles):
                injB = mk_injB(i + 1) if i + 1 < len(all_tiles) else None
                injA = mk_injA(i + 1) if i + 1 < len(all_tiles) else None
                injE = None
                if i + 1 < len(all_tiles) and all_tiles[i + 1][1] == "s":
                    injE = mk_injE(i + 1)
                pg = postg_for(i)
                if kind == "m":
                    grp_main(lvl, a, P, ctx[i], injectB=injB, injectA=injA,
                             injectE=injE, postg=pg)
                else:
                    grp_sliver(lvl, a, P, ctx[i], injectB=injB, injectA=injA,
                               postg=pg, se_pre=se_pre.pop(i, None))

    nc.compile()
    return nc


_NC_CACHE = None


def _get_nc():
    global _NC_CACHE
    if _NC_CACHE is None:
        _NC_CACHE = _build()
    return _NC_CACHE


def _host_prep(inp):
    """Build per-core input dicts from the full (B,C,H,W) f32 array."""
    aff = inp[:, : 2 * NOFF]
    emb = inp[:, 2 * NOFF:]

    # shift matrices: S[lvl*3+t][p + t*d, p] = 1
    sm = np.zeros((6, 128, 128), dtype=np.float32)
    for lvl in range(2):
        d = 3 ** lvl
        for t in range(3):
            for p in range(128):
                if p + t * d < 128:
                    sm[lvl * 3 + t, p + t * d, p] = 1.0
    sm = sm.astype(ml_dtypes.bfloat16)

    # stacked sliver matrices: SS[lvl*2+s][b*K0 + p + t_b*d, p] = 1
    ss = np.zeros((4, 128, 32), dtype=np.float32)
    for lvl, (a_, P_) in ((0, SL0), (1, SL1)):
        d = 3 ** lvl
        K0 = P_ + 2 * d
        for s, js in enumerate(SLIV[lvl]):
            for b, j in enumerate(js):
                t = BASE[j][0] + 1
                for p in range(P_):
                    ss[lvl * 2 + s, b * K0 + p + t * d, p] = 1.0
    ss = ss.astype(ml_dtypes.bfloat16)

    cols_e = np.clip(np.arange(-1, 513), 0, W - 1)

    maps = []
    for b in range(B):
        affb16 = aff[b].astype(np.float16)
        embb16 = emb[b].astype(ml_dtypes.bfloat16)
        for h in range(2):
            base = h * HALF
            # affp[lvl, j, r] = aff ch 9*lvl+j at core row (r - 6 - t_j * d)
            affp = np.empty((2, NOFF, RA, W), np.float16)
            for lvl in range(2):
                d = 3 ** lvl
                for t in range(3):
                    rt = np.clip(base + np.arange(-6, 262) - t * d, 0, H - 1)
                    affp[lvl, 3 * t: 3 * t + 3] = affb16[
                        NOFF * lvl + 3 * t: NOFF * lvl + 3 * t + 3][:, rt, :]
            affp = np.ascontiguousarray(affp.transpose(0, 2, 1, 3))
            re = np.clip(base + np.arange(-4, 262), 0, H - 1)
            ep = np.ascontiguousarray(
                embb16[:, re][:, :, cols_e].transpose(1, 0, 2))
            rf = np.zeros((32, 2, 32), np.float32)
            for p in range(3):
                rf[3 if h == 0 else p, 0, p] = 1.0
                rf[14 if h == 1 else 15 + p, 1, p] = 1.0
            maps.append({"aff": affp, "e0": ep, "sm": sm, "ss": ss,
                         "rf": rf.astype(ml_dtypes.bfloat16)})
    return maps


def kernel(input):
    inp = np.asarray(input, dtype=np.float32)
    nc = _get_nc()
    maps = _host_prep(inp)
    res = run_bass_kernel_spmd(nc, maps, core_ids=list(range(NCORE)))
    full = np.empty((B, E, H, W), dtype=np.float32)
    k = 0
    for b in range(B):
        for h in range(2):
            out = np.asarray(res.results[k]["out"])  # [HALF, E, W]
            full[b, :, h * HALF: (h + 1) * HALF, :] = out.transpose(1, 0, 2)
            k += 1
    return full

